# revision 26
# baseline (speedup 1.0000x reference)
"""Trainium2 Bass kernel: GQA attention layer with RoPE + int8 quant-dequant KV.

Tensor-parallel over heads across 8 NeuronCores: core c owns q-heads
[4c, 4c+4) and kv-head c.  Each core computes its partial output
y_c = attn_out_local @ wo_local.T; the host sums the 8 partials.

v2.4 pipeline (per core), software-pipelined over 512-token query tiles:
  iteration J runs   attention+wo  for tile J-1   (uses qts/kT/v from J-1)
  interleaved with   proj+RoPE+qd  for tile J     (produces them for J)
The attention j-loop is ACT(exp)-bound, so the q-projection matmuls are
spliced INTO it at matmul granularity ("fillers"); the k-quant transposes
are woven between wo column-chunks so their DVE chains hide under wo.

Softmax: unnormalized exp; per-block e accumulates into an SBUF esum on
DVE (bf16 partials - per-partition rounding averages out in the f32
partition-sum), one all-ones matmul broadcasts the denominator, fast
custom-DVE reciprocal, one tensor_tensor normalizes the AV accumulator.
"""
import math
import numpy as np
from contextlib import ExitStack

import concourse.bass as bass
import concourse.bacc as bacc
import concourse.mybir as mybir
import concourse.tile as tile
from concourse.bass_utils import run_bass_kernel_spmd
from concourse.masks import make_identity

F32 = mybir.dt.float32
BF16 = mybir.dt.bfloat16
AF = mybir.ActivationFunctionType
ALU = mybir.AluOpType
AX = mybir.AxisListType

MAGIC = 1.5 * 2.0**23  # fp32 RNE integer-rounding magic constant
NCORES = 8


def build_nc(S=2048, D=4096, HL=4, QT=512, repeat=1):
    """Per-core Bass graph. HL local q heads, 1 local kv head."""
    DT = D // 128    # contraction tiles
    NQ = S // QT     # query tiles
    DB = QT // 128   # 128-blocks per query tile
    KB = S // 128    # total k blocks
    NDC = D // 512   # wo output column tiles
    ISCL = 1.0 / math.sqrt(128.0)

    nc = bacc.Bacc("TRN2")
    hsN = nc.declare_dram_parameter("hsN", [NQ, 128, DT * QT], BF16, isOutput=False)
    wqN = nc.declare_dram_parameter("wqN", [128, DT * HL * 128], BF16, isOutput=False)
    wkN = nc.declare_dram_parameter("wkN", [128, DT * 128], BF16, isOutput=False)
    wvN = nc.declare_dram_parameter("wvN", [128, DT * 128], BF16, isOutput=False)
    woT = nc.declare_dram_parameter("woT", [HL * 128, D], BF16, isOutput=False)
    cosT = nc.declare_dram_parameter("cosT", [128, S], BF16, isOutput=False)
    sinT = nc.declare_dram_parameter("sinT", [128, S], BF16, isOutput=False)
    triT = nc.declare_dram_parameter("triT", [128, 128], BF16, isOutput=False)
    rotT = nc.declare_dram_parameter("rotT", [128, 128], BF16, isOutput=False)
    y = nc.declare_dram_parameter("y", [S, D], BF16, isOutput=True)

    with tile.TileContext(nc) as tc, ExitStack() as ctx:
        const = ctx.enter_context(tc.tile_pool(name="const", bufs=1))
        persist = ctx.enter_context(tc.tile_pool(name="persist", bufs=1))
        hs_pool = ctx.enter_context(tc.tile_pool(name="hs", bufs=6))
        qpool = ctx.enter_context(tc.tile_pool(name="qpool", bufs=2 * HL))
        kro_pool = ctx.enter_context(tc.tile_pool(name="krop", bufs=2))
        raw_pool = ctx.enter_context(tc.tile_pool(name="rawp", bufs=2))
        expp = ctx.enter_context(tc.tile_pool(name="expp", bufs=3))
        apool = ctx.enter_context(tc.tile_pool(name="apool", bufs=2 * HL + 1))
        recp = ctx.enter_context(tc.tile_pool(name="recp", bufs=2))
        esump = ctx.enter_context(tc.tile_pool(name="esump", bufs=2))
        ysb_pool = ctx.enter_context(tc.tile_pool(name="ysb", bufs=2))
        rows = ctx.enter_context(tc.tile_pool(name="rows", bufs=8))
        qdw = ctx.enter_context(tc.tile_pool(name="qdw", bufs=4))
        # PSUM: exactly 8 banks.  pp_proj doubles as the transpose pool in
        # kv_finish (proj is idle there); pp_kv holds the single k/v proj
        # accumulator (k and v run as separate passes).
        pp_proj = ctx.enter_context(tc.tile_pool(name="pproj", bufs=2, space="PSUM"))
        pp_kv = ctx.enter_context(tc.tile_pool(name="pkv", bufs=1, space="PSUM"))
        pp_s = ctx.enter_context(tc.tile_pool(name="ps", bufs=2, space="PSUM"))
        pp_o = ctx.enter_context(tc.tile_pool(name="po", bufs=2, space="PSUM"))
        pp_d = ctx.enter_context(tc.tile_pool(name="pd", bufs=1, space="PSUM"))

        # ---- persistent weights / constants ----
        # DMA issue order pairs wq quarters with the hs chunks the first
        # projection pass consumes alongside them (startup is DMA-bound).
        wq_sb = persist.tile([128, DT * HL * 128], BF16, name="wq", tag="wq")
        hs_tiles = {}
        HSC = DT // 4  # d-tiles per hs DMA chunk

        def dma_hs_chunk(J, c, ts):
            t = hs_pool.tile([128, HSC * QT], BF16, name="hs", tag="hs")
            nc.sync.dma_start(out=t[:],
                              in_=hsN[J, :, c * HSC * QT:(c + 1) * HSC * QT])
            ts.append(t)

        def dma_hs(J):
            ts = []
            for c in range(4):
                dma_hs_chunk(J, c, ts)
            hs_tiles[J] = ts

        hq = DT * HL * 128 // 4
        ts0 = []
        for i in range(4):
            if i == 0:
                t = hs_pool.tile([128, HSC * QT], BF16, name="hs", tag="hs")
                for half in range(2):
                    wsl = slice(half * hq // 2, (half + 1) * hq // 2)
                    nc.sync.dma_start(out=wq_sb[:, wsl], in_=wqN[:, wsl])
                    hsl = slice(half * HSC * QT // 2, (half + 1) * HSC * QT // 2)
                    nc.sync.dma_start(out=t[:, hsl], in_=hsN[0, :, hsl])
                ts0.append(t)
            else:
                nc.sync.dma_start(out=wq_sb[:, i * hq:(i + 1) * hq],
                                  in_=wqN[:, i * hq:(i + 1) * hq])
                dma_hs_chunk(0, i, ts0)
        hs_tiles[0] = ts0
        cos_sb = const.tile([128, S], BF16, name="cos", tag="cos")
        nc.sync.dma_start(out=cos_sb[:], in_=cosT[:])
        sin_sb = const.tile([128, S], BF16, name="sin", tag="sin")
        nc.sync.dma_start(out=sin_sb[:], in_=sinT[:])
        tri_sb = const.tile([128, 128], BF16, name="tri", tag="tri")
        nc.sync.dma_start(out=tri_sb[:], in_=triT[:])
        rot_sb = const.tile([128, 128], BF16, name="rot", tag="rot")
        nc.sync.dma_start(out=rot_sb[:], in_=rotT[:])
        wk_sb = persist.tile([128, DT * 128], BF16, name="wk", tag="wk")
        nc.sync.dma_start(out=wk_sb[:], in_=wkN[:])
        wv_sb = persist.tile([128, DT * 128], BF16, name="wv", tag="wv")
        nc.sync.dma_start(out=wv_sb[:], in_=wvN[:])
        wo_sb = persist.tile([128, HL, D], BF16, name="wo", tag="wo")
        for hb in range(HL):
            nc.sync.dma_start(out=wo_sb[:, hb, :], in_=woT[hb * 128:(hb + 1) * 128, :])

        identb = const.tile([128, 128], BF16, name="identb", tag="identb")
        make_identity(nc, identb[:])
        ones128 = const.tile([128, 128], BF16, name="ones", tag="ones")
        nc.vector.memset(ones128[:], 1.0)
        zbias = const.tile([128, 1], F32, name="zbias", tag="zbias")
        nc.vector.memset(zbias[:], 0.0)
        magicb = const.tile([128, 1], F32, name="magicb", tag="magicb")
        nc.vector.memset(magicb[:], MAGIC)
        nmagicb = const.tile([128, 1], F32, name="nmagicb", tag="nmagicb")
        nc.vector.memset(nmagicb[:], -MAGIC)

        kT_all = persist.tile([128, S], BF16, name="kT", tag="kT")
        v_nat = persist.tile([128, KB, 128], BF16, name="vnat", tag="vnat")

        qts_of = {}       # J -> [qt tiles]
        ats_of = {}       # W -> [a_t tiles]

        def proj_q_fillers(J, h):
            """One-matmul emitters for q-head h of tile J (spliced into the
            attention j-loop to fill its ACT-bound PE bubbles)."""
            state = {}

            def mk(d):
                def emit():
                    if "pq" not in state:
                        state["pq"] = pp_proj.tile([128, QT], F32,
                                                   name="pq", tag="pq")
                    hs_t = hs_tiles[J][d // HSC]
                    hsl = (d % HSC) * QT
                    nc.tensor.matmul(state["pq"][:],
                                     wq_sb[:, d * HL * 128 + h * 128:
                                           d * HL * 128 + (h + 1) * 128],
                                     hs_t[:, hsl:hsl + QT],
                                     start=(d == 0), stop=(d == DT - 1))
                return emit

            return [mk(d) for d in range(DT)], state

        def rope_copy(state):
            """Evacuate the projection PSUM to SBUF bf16 (frees the bank)."""
            raw = raw_pool.tile([128, QT], BF16, name="raw", tag="raw")
            nc.vector.tensor_copy(raw[:], state["pq"][:])
            state["raw"] = raw

        def rope_rest(state, J, cons_pool, out_pool, out_tag, sink):
            """rot matmul + cos/sin combine; emitted a few PE slots after
            rope_copy so the rot matmul never waits on the DVE copy."""
            qsl = slice(J * QT, (J + 1) * QT)
            raw = state["raw"]
            rot_ps = cons_pool.tile([128, QT], F32, name="rps",
                                    tag="pq" if cons_pool is pp_proj else "pkv")
            nc.tensor.matmul(rot_ps[:], rot_sb[:], raw[:], start=True, stop=True)
            tmp = raw_pool.tile([128, QT], BF16, name="tmp", tag="tmp")
            nc.vector.tensor_tensor(out=tmp[:], in0=raw[:], in1=cos_sb[:, qsl],
                                    op=ALU.mult)
            t2 = raw_pool.tile([128, QT], BF16, name="t2", tag="t2")
            nc.vector.tensor_tensor(out=t2[:], in0=rot_ps[:], in1=sin_sb[:, qsl],
                                    op=ALU.mult)
            out_t = out_pool.tile([128, QT], BF16, name=out_tag, tag=out_tag)
            nc.vector.tensor_tensor(out=out_t[:], in0=tmp[:], in1=t2[:],
                                    op=ALU.add)
            sink(out_t)

        def rope(p_ps, J, cons_pool, out_pool, out_tag):
            st = {"pq": p_ps}
            rope_copy(st)
            res = []
            rope_rest(st, J, cons_pool, out_pool, out_tag, res.append)
            return res[0]

        def proj_kv_pass(J, w_sb):
            """32-matmul k or v projection into the single pp_kv bank."""
            p = pp_kv.tile([128, QT], F32, name="pkv", tag="pkv")
            for d in range(DT):
                hs_t = hs_tiles[J][d // HSC]
                hsl = (d % HSC) * QT
                nc.tensor.matmul(p[:], w_sb[:, d * 128:(d + 1) * 128],
                                 hs_t[:, hsl:hsl + QT],
                                 start=(d == 0), stop=(d == DT - 1))
            return p

        def qd_block(src_ps, out_ap):
            """int8 quant-dequant of one natural [tok, dh] block in PSUM.

            DVE: absmax + scale + recip; ACT: magic-round affines (the
            subtract-magic runs at scale=1 so Sterbenz keeps it exact)."""
            amax = rows.tile([128, 1], F32, name="amax", tag="row")
            nc.vector.tensor_reduce(out=amax[:], in_=src_ps, axis=AX.X,
                                    op=ALU.max, apply_absolute_value=True)
            scl = rows.tile([128, 1], F32, name="scl", tag="row")
            nc.vector.tensor_scalar(out=scl[:], in0=amax[:],
                                    scalar1=1.0 / 127.0, scalar2=1e-8,
                                    op0=ALU.mult, op1=ALU.max)
            inv = rows.tile([128, 1], F32, name="inv", tag="row")
            nc.vector.reciprocal(inv[:], scl[:])
            xs = qdw.tile([128, 128], F32, name="xs", tag="xs")
            nc.scalar.activation(xs[:], src_ps, AF.Identity,
                                 bias=magicb[:], scale=inv[:])
            xr = qdw.tile([128, 128], F32, name="xr", tag="xr")
            nc.scalar.activation(xr[:], xs[:], AF.Identity,
                                 bias=nmagicb[:], scale=1.0)
            nc.vector.tensor_scalar(out=out_ap, in0=xr[:], scalar1=scl[:],
                                    scalar2=None, op0=ALU.mult)

        tr_state = {"cycle": None, "i": 0}

        def tr_alloc():
            cyc = tr_state["cycle"]
            if cyc is not None:
                pool, tag = cyc[tr_state["i"] % len(cyc)]
                tr_state["i"] += 1
            else:
                pool, tag = pp_proj, "pq"
            t = pool.tile([128, 1024], BF16, name="tr", tag=tag)
            return t[:, 0:128]

        def attn_head_mm(W, h, filler, fin_cb):
            """Causal attention j-loop; pops 2 proj fillers per j; runs
            fin_cb (previous head's finalize) after the second j."""
            nkb = (W + 1) * DB
            out_ps = pp_o.tile([128, QT], F32, name="outp", tag="po")
            esum = esump.tile([128, QT], BF16, name="esum", tag="esum")
            qt = qts_of[W][h]
            for j in range(nkb):
                r = j - W * DB
                qoff = max(r, 0) * 128
                w = QT - qoff
                # full-bank PSUM tile (sub-bank tiles can share a physical
                # bank -> fatal PE-write/engine-read collisions on HW)
                s_full = pp_s.tile([128, QT], F32, name="sps", tag="ps")
                s_ps = s_full[:, 0:w]
                nc.tensor.matmul(s_ps, kT_all[:, j * 128:(j + 1) * 128],
                                 qt[:, qoff:QT], start=True, stop=True)
                e_sb = expp.tile([128, w], BF16, name="esb", tag="e")
                nc.scalar.activation(e_sb[:], s_ps, AF.Exp,
                                     bias=zbias[:], scale=ISCL)
                if r >= 0:
                    nc.vector.tensor_tensor(out=e_sb[:, 0:128], in0=e_sb[:, 0:128],
                                            in1=tri_sb[:], op=ALU.mult)
                first, last = j == 0, j == nkb - 1
                if first:
                    nc.vector.tensor_copy(esum[:], e_sb[:])
                else:
                    nc.vector.tensor_tensor(out=esum[:, qoff:QT],
                                            in0=esum[:, qoff:QT],
                                            in1=e_sb[:], op=ALU.add)
                nc.tensor.matmul(out_ps[:, qoff:QT], v_nat[:, j, :], e_sb[:],
                                 start=first, stop=last)
                for f in (next(filler, None), next(filler, None)):
                    if f is not None:
                        f()
                if j == 1 and fin_cb is not None:
                    fin_cb()
                    fin_cb = None
            if fin_cb is not None:
                fin_cb()
            return out_ps, esum

        def attn_head_fin(out_ps, esum, ats):
            """Reduce esum across partitions (broadcast), recip, normalize."""
            den_ps = pp_d.tile([128, QT], F32, name="denp", tag="pd")
            nc.tensor.matmul(den_ps[:], ones128[:], esum[:], start=True, stop=True)
            rec = recp.tile([128, QT], F32, name="rec", tag="rec")
            nc.vector.reciprocal_approx_fast(rec[:], den_ps[:])
            a_t = apool.tile([128, QT], BF16, name="at", tag="at")
            nc.vector.tensor_tensor(out=a_t[:], in0=out_ps[:], in1=rec[:],
                                    op=ALU.mult)
            ats.append(a_t)

        def wo_unit(W, t, dc, ats, st, pool, tag):
            """One (t, dc) output group: 4 matmuls + evac (+ eager DMA)."""
            t_sl = slice(t * 128, (t + 1) * 128)
            if dc == 0:
                st["ysb"] = ysb_pool.tile([128, D], BF16, name="ysb", tag="ysb")
            y_sb = st["ysb"]
            row = W * QT + t * 128
            y_ps = pool.tile([128, 512], F32, name="yps", tag=tag)
            for hb in range(HL):
                nc.tensor.matmul(y_ps[:], ats[hb][:, t_sl],
                                 wo_sb[:, hb, dc * 512:(dc + 1) * 512],
                                 start=(hb == 0), stop=(hb == HL - 1))
            dsl = slice(dc * 512, (dc + 1) * 512)
            if dc % 2 == 0:
                nc.vector.tensor_copy(y_sb[:, dsl], y_ps[:])
            else:
                nc.scalar.activation(y_sb[:, dsl], y_ps[:], AF.Copy)
                csl = slice((dc - 1) * 512, (dc + 1) * 512)
                nc.sync.dma_start(out=y[row:row + 128, csl],
                                  in_=y_sb[:, csl])

        def wo_chunk(W, t, ats, pool=None, tag=None):
            st = {}
            for dc in range(NDC):
                wo_unit(W, t, dc, ats, st, pool or pp_o, tag or "po")

        def wo_fill_units(W, ats):
            """All 32 wo units of tile W as filler callables (y_ps from the
            proj pool, which is idle in the final iteration)."""
            sts = [dict() for _ in range(DB)]
            return [
                (lambda t=t, dc=dc: wo_unit(W, t, dc, ats, sts[t],
                                            pp_proj, "pq"))
                for t in range(DB) for dc in range(NDC)
            ]

        def bootstrap_pb(J):
            """No-attention iteration (J=0 / rep start): run all 6 projection
            accumulators chunk-major so PE keeps pace with the hs/wq DMA
            stream instead of waiting per-head.  Borrows the (idle)
            attention PSUM banks."""
            pools = [pp_proj, pp_proj, pp_s, pp_o]
            tags = ["pq", "pq", "ps", "po"]
            pqs = [pools[h].tile([128, QT], F32, name="pq0", tag=tags[h])
                   for h in range(HL)]
            pk = pp_kv.tile([128, QT], F32, name="pk0", tag="pkv")
            pv = pp_d.tile([128, QT], F32, name="pv0", tag="pd")
            for d in range(DT):
                hs_t = hs_tiles[J][d // HSC]
                hsl = (d % HSC) * QT
                first, last = d == 0, d == DT - 1
                for h in range(HL):
                    nc.tensor.matmul(pqs[h][:],
                                     wq_sb[:, d * HL * 128 + h * 128:
                                           d * HL * 128 + (h + 1) * 128],
                                     hs_t[:, hsl:hsl + QT],
                                     start=first, stop=last)
                nc.tensor.matmul(pk[:], wk_sb[:, d * 128:(d + 1) * 128],
                                 hs_t[:, hsl:hsl + QT], start=first, stop=last)
                nc.tensor.matmul(pv[:], wv_sb[:, d * 128:(d + 1) * 128],
                                 hs_t[:, hsl:hsl + QT], start=first, stop=last)
            sts = [{"pq": p} for p in pqs]
            kst = {"pq": pk}
            kres = []
            rope_copy(sts[0])
            rope_copy(sts[1])
            rope_rest(sts[0], J, pp_proj, qpool, "qt", qts_of[J].append)
            rope_copy(sts[2])
            rope_rest(sts[1], J, pp_proj, qpool, "qt", qts_of[J].append)
            rope_copy(sts[3])
            rope_rest(sts[2], J, pp_proj, qpool, "qt", qts_of[J].append)
            rope_copy(kst)
            rope_rest(sts[3], J, pp_proj, qpool, "qt", qts_of[J].append)
            rope_rest(kst, J, pp_proj, kro_pool, "krope", kres.append)
            vraw = kro_pool.tile([128, QT], BF16, name="vraw", tag="vraw")
            nc.scalar.activation(vraw[:], pv[:], AF.Copy)
            return kres[0], vraw

        # k-quant pipeline pieces, interleaved with wo chunks
        def k_tr(t, krope, kq_nats):
            tr_ps = tr_alloc()
            nc.tensor.transpose(tr_ps, krope[:, t * 128:(t + 1) * 128], identb[:])
            kq = qdw.tile([128, 128], BF16, name="kq", tag="kq")
            qd_block(tr_ps, kq[:])
            kq_nats.append(kq)

        def k_tr2(J, t, kq_nats):
            col = slice(J * QT + t * 128, J * QT + (t + 1) * 128)
            tr2_ps = tr_alloc()
            nc.tensor.transpose(tr2_ps, kq_nats[t][:], identb[:])
            nc.vector.tensor_copy(kT_all[:, col], tr2_ps)

        def v_tr(J, t, vraw):
            tr_ps = tr_alloc()
            nc.tensor.transpose(tr_ps, vraw[:, t * 128:(t + 1) * 128], identb[:])
            qd_block(tr_ps, v_nat[:, J * DB + t, :])

        # ---- software-pipelined main loop ----
        pb_seq = [J for _ in range(repeat) for J in range(NQ)]
        pb_pos = 0
        for rep in range(repeat):
            for J in range(NQ + 1):
                W = J - 1
                has_pb = J < NQ
                has_wb = W >= 0
                if has_pb:
                    qts_of[J] = []
                ats = []
                ats_of[W] = ats
                # final iteration: no projection work exists, so the
                # deferred wo of tile W-1 becomes the attention filler
                wof = (wo_fill_units(W - 1, ats_of[W - 1])
                       if (not has_pb and has_wb and W - 1 in ats_of)
                       else [])
                pend = None
                prev_state = None
                qsink = (qts_of[J].append) if has_pb else None
                for h in range(HL):
                    if has_pb:
                        fq, pq_state = proj_q_fillers(J, h)
                    else:
                        fq = []
                        for u in wof[h * 8:(h + 1) * 8]:
                            fq += [u, None, None, None]
                        pq_state = None
                    fillers = fq
                    if prev_state is not None:
                        ps = prev_state
                        cpf = (lambda s=ps: rope_copy(s))
                        rsf = (lambda s=ps: rope_rest(s, J, pp_proj,
                                                      qpool, "qt", qsink))
                        fillers = [cpf] + fq[0:5] + [rsf] + fq[5:]
                    filler = iter(fillers)
                    if has_wb:
                        fin_cb = (lambda p=pend: attn_head_fin(*p, ats)) \
                            if pend is not None else None
                        pend = attn_head_mm(W, h, filler, fin_cb)
                    for f in filler:   # drain unconsumed emitters
                        if f is not None:
                            f()
                    prev_state = pq_state if has_pb else None
                if has_pb:
                    rope_copy(prev_state)
                    if pb_pos + 1 < len(pb_seq):
                        dma_hs(pb_seq[pb_pos + 1])
                    pb_pos += 1
                    pk = proj_kv_pass(J, wk_sb)
                if has_wb and pend is not None:
                    attn_head_fin(*pend, ats)
                    pend = None
                emit_wo = has_wb and J != NQ - 1
                deep_tr = has_pb and (not emit_wo)
                if has_pb:
                    rope_rest(prev_state, J, pp_proj, qpool, "qt", qsink)
                    krope = rope(pk, J, pp_kv, kro_pool, "krope")
                    pv = proj_kv_pass(J, wv_sb)
                    vraw = kro_pool.tile([128, QT], BF16, name="vraw", tag="vraw")
                    nc.scalar.activation(vraw[:], pv[:], AF.Copy)
                    kq_nats = []
                    k_tr(0, krope, kq_nats)
                    k_tr(1, krope, kq_nats)
                    if emit_wo:
                        wo_chunk(W, 0, ats)
                    k_tr2(J, 0, kq_nats)
                    k_tr(2, krope, kq_nats)
                    if emit_wo:
                        wo_chunk(W, 1, ats)
                    k_tr2(J, 1, kq_nats)
                    k_tr(3, krope, kq_nats)
                    if emit_wo:
                        wo_chunk(W, 2, ats)
                    k_tr2(J, 2, kq_nats)
                    v_tr(J, 0, vraw)
                    if emit_wo:
                        wo_chunk(W, 3, ats)
                    k_tr2(J, 3, kq_nats)
                    v_tr(J, 1, vraw)
                    v_tr(J, 2, vraw)
                    v_tr(J, 3, vraw)
                    tr_state["cycle"] = None
                elif has_wb:
                    for t in range(DB):
                        wo_chunk(W, t, ats)

    nc.compile()
    return nc


def host_inputs(hidden_states, wq, wk, wv, wo, position_ids,
                S=2048, D=4096, HL=4, QT=512, ncores=NCORES):
    """Shard + preprocess inputs -> per-core in_maps (bf16, SBUF layouts)."""
    import ml_dtypes
    bf = ml_dtypes.bfloat16
    DT = D // 128
    NQ = S // QT

    hs = np.asarray(hidden_states, np.float32)[0]          # [S, D]
    hsT = hs.T                                             # [D, S]
    # hsN[J][p, d*QT + c] = hsT[d*128+p, J*QT+c]
    hsN = np.ascontiguousarray(
        hsT.reshape(DT, 128, NQ, QT).transpose(2, 1, 0, 3)
    ).reshape(NQ, 128, DT * QT).astype(bf)

    pos = np.asarray(position_ids)[0].astype(np.float32)
    inv_freq = (1.0 / (10000.0 ** (np.arange(0, 128, 2, dtype=np.float32) / 128.0)))
    freqs = pos[:, None] * inv_freq[None, :]
    emb = np.concatenate([freqs, freqs], axis=1)           # [S, 128]
    cosT = np.ascontiguousarray(np.cos(emb).T).astype(bf)
    sinT = np.ascontiguousarray(np.sin(emb).T).astype(bf)

    kk = np.arange(128)[:, None]
    qq = np.arange(128)[None, :]
    triT = (kk <= qq).astype(bf)                           # [k, q] causal block

    rotT = np.zeros((128, 128), np.float32)
    idx = np.arange(64)
    rotT[idx, idx + 64] = 1.0
    rotT[idx + 64, idx] = -1.0
    rotT = rotT.astype(bf)

    wq = np.asarray(wq, np.float32)
    wk = np.asarray(wk, np.float32)
    wv = np.asarray(wv, np.float32)
    wo = np.asarray(wo, np.float32)

    def pack_w(wslice, ncols):
        # wslice [ncols_out, D] -> [128, DT*ncols]: [p, d*ncols + c] = w.T[d*128+p, c]
        wt = wslice.T                                      # [D, ncols]
        return np.ascontiguousarray(
            wt.reshape(DT, 128, ncols).transpose(1, 0, 2)
        ).reshape(128, DT * ncols).astype(bf)

    in_maps = []
    qh = HL * 128
    for c in range(ncores):
        wqN_c = pack_w(wq[c * qh:(c + 1) * qh, :], qh)
        wkN_c = pack_w(wk[c * 128:(c + 1) * 128, :], 128)
        wvN_c = pack_w(wv[c * 128:(c + 1) * 128, :], 128)
        woT_c = np.ascontiguousarray(wo[:, c * qh:(c + 1) * qh].T).astype(bf)
        in_maps.append({
            "hsN": hsN, "wqN": wqN_c, "wkN": wkN_c, "wvN": wvN_c,
            "woT": woT_c, "cosT": cosT, "sinT": sinT, "triT": triT,
            "rotT": rotT,
        })
    return in_maps


_NC_CACHE = {}
COMPUTE = "bf16"


def kernel(hidden_states, wq, wk, wv, wo, position_ids):
    import time
    B, S, D = hidden_states.shape
    in_maps = host_inputs(hidden_states, wq, wk, wv, wo, position_ids, S=S, D=D)
    key = (S, D, 1)
    if key not in _NC_CACHE:
        _NC_CACHE[key] = build_nc(S=S, D=D, repeat=1)
    nc = _NC_CACHE[key]
    res = None
    for attempt in range(3):
        # A freshly loaded NEFF's first execution can transiently hit a
        # wedged-device NRT error (which itself resets the device); retry.
        try:
            res = run_bass_kernel_spmd(nc, in_maps,
                                       core_ids=list(range(NCORES)),
                                       trace=False)
            break
        except Exception:
            if attempt == 2:
                raise
            time.sleep(3.0)
    y = np.zeros((S, D), np.float64)
    for c in range(NCORES):
        y += res.results[c]["y"].astype(np.float64)
    return y.astype(np.float32)[None]
